# revision 28
# baseline (speedup 1.0000x reference)
"""MQA attention kernel for Trainium2 (8 NeuronCores, Bass/Tile).

Problem: Q [2,16,2048,64], K/V [2,1,2048,64] fp32, out = softmax(QK^T/8) V.

Sharding: 32 (batch, head) pairs over 8 cores -> 4 heads per core; each core
gets one batch's K/V (replicated across the 4 cores of that batch).

Per-core algorithm (S^T orientation so softmax reduction lands on the free dim
and PV needs no transposition of P):
  - inputs are cast to bf16 on the DVE, then K^T/Q^T are built with bf16 PE
    transposes (half the cost of fp32 ones) into [128, S] bf16 tiles with
    rows 64-127 zeroed (a 64-partition stationary flips the PE into a slow
    half-width tile mode, so the full 128-row contraction is kept).
  - S^T[j, q] = (K Q^T) computed in bf16 matmuls (1 cyc/row), one PSUM bank
    per 128-row j-chunk.
  - exp(s/8) fused with PSUM->SBUF evacuation in j-chunk groups of
    3/3/3/3/2/2 per q-block: the four 3-groups on the scalar engine
    (ACTIVATE, output bf16; no max subtraction: scores/8 ~ N(0,1), exp
    never overflows fp32), the two 2-groups on the DVE via the Schraudolph
    bit trick (see EXP_OFF_GIS below).
  - PV uses bf16 V augmented with a ones column: one accumulating matmul
    chain yields both O^T = V'^T P^T and the softmax denominators (row 64).
  - O'^T transposed back with PE, normalized with DVE reciprocal+mul, DMA out.

The q-rows are processed in an interleaved order (partition p holds rows
16p+c) so every DMA moves contiguous 4KB runs; the same rearrange on the
output store undoes the permutation.

Measured on trn2 (NTFF profile, max over 8 cores): 152.2us end-to-end
(baseline 180.1us); the PE matmul stream (QK + PV + transposes, ~127us at
80% occupancy) is the roofline, with the scalar engine's exp stream (~99us)
and the DVE (~95us) packed underneath. Rel err vs the fp32 jax reference:
8.1e-3 (bf16 matmuls + Schraudolph exp on 4/16 chunks; gate is 2e-2).

Scheduling:
  - a 1-element dummy exp issues first so the ACT table load (~1.3us)
    happens during the input DMAs.
  - zero-fill memsets for the bf16 K^T/Q^T dead rows are chunked on gpsimd
    so the identity build isn't stuck behind them.
  - startup DMAs are split into 4-chunk pieces; the first K^T/Q^T transpose
    group runs after the first piece, and q-block 0's QK groups interleave
    with the remaining transpose groups.
  - each q-block's PV chain is split: chunks 0-11 right after its QK groups,
    chunks 12-15 after the NEXT q-block's first QK group, so the scalar
    engine never waits for PV+epilogue at block boundaries; the LAST
    q-block's PV chunks are interleaved with its exp groups instead.
  - the epilogue batches its 4 output transposes, then one strided
    reciprocal + 4 multiplies, avoiding a serialized PE<->DVE ping-pong.
  - head h+1's Q^T prep is spread across head h's q-blocks 1-3.
PSUM budget (8 banks): 2x3-bank rotating score slots + 1 PV accumulator +
1 transpose-staging bank (the PV slot doubles as a second staging bank).
"""

import numpy as np

import concourse.bass as bass
import concourse.mybir as mybir
import concourse.tile as tile
from concourse import bacc
from concourse.bass_utils import run_bass_kernel_spmd
from concourse.masks import make_identity

B, H, S, D = 2, 16, 2048, 64
N_CORES = 8
HPC = (B * H) // N_CORES  # heads per core = 4
P = 128
NJ = S // P               # 16 key chunks of 128
QB = 512                  # queries per block
NQB = S // QB             # 4 q-blocks per head
SCALE = 1.0 / float(D) ** 0.5
F32 = mybir.dt.float32
BF16 = mybir.dt.bfloat16
GROUPS = [3, 3, 3, 3, 2, 2]  # j-chunks per exp group
GCUM = [sum(GROUPS[: i + 1]) for i in range(len(GROUPS))]
PV_SPLIT = GCUM[3]           # PV chunks emitted with their own q-block (12)
N_UNITS = HPC * NQB
# Groups 4-5 (chunks 12-15) have their exp offloaded from the scalar engine
# to the DVE via the Schraudolph bit trick in bf16: int16(round(a*s + b))
# reinterpreted as bf16 is exp(s*SCALE) with ~1.8% RMS error (validated
# end-to-end under the 2e-2 budget). One fused DVE tensor_scalar per group.
# These chunks' PV matmuls are the ones deferred into the next unit, so the
# DVE's extra latency is off the critical path, and the scalar engine's
# remaining 4 ACTIVATEs per q-block drop it well below the PE's pace.
EXP_OFF_GIS = (4, 5)
EXP_A = (2.0 ** 7 / float(np.log(2.0))) * SCALE
EXP_B = 127.0 * 2 ** 7 - 7.5

_CACHED = {}
DEFAULT_CFG = {}


def _build_module(reps=1, **cfg):
    nc = bacc.Bacc(None)
    q = nc.dram_tensor("q", [HPC, S, D], F32, kind="ExternalInput")
    k = nc.dram_tensor("k", [S, D], F32, kind="ExternalInput")
    v = nc.dram_tensor("v", [S, D], F32, kind="ExternalInput")
    o = nc.dram_tensor("o", [HPC, S, D], F32, kind="ExternalOutput")

    with tile.TileContext(nc) as tc:
        with tc.tile_pool(name="const", bufs=1) as cpool:
            identity = cpool.tile([P, P], F32)
            make_identity(nc, identity)
            identity_bf = cpool.tile([P, P], BF16, name="id_bf")
            make_identity(nc, identity_bf)

            kT = cpool.tile([P, S], BF16)
            vp = cpool.tile([P, NJ, D + 1], BF16)
            qT_tiles = [cpool.tile([P, S], BF16, name=f"qT{i}") for i in range(2)]
            # zero the dead contraction rows in column chunks so the first
            # QK groups aren't stuck behind one long memset
            for pc in range(4):
                cs = slice(QB * pc, QB * (pc + 1))
                nc.gpsimd.memset(kT[D:P, cs], 0.0)
                nc.gpsimd.memset(qT_tiles[0][D:P, cs], 0.0)
            for pc in range(2):
                nc.gpsimd.memset(qT_tiles[1][D:P, S // 2 * pc : S // 2 * (pc + 1)], 0.0)
            nc.gpsimd.memset(vp[:, :, D], 1.0)

            for rep in range(reps):
                _trace_body(nc, tc, q, k, v, o, identity, identity_bf, kT, vp, qT_tiles, **cfg)
    nc.compile()
    return nc


def _trace_body(nc, tc, q, k, v, o, identity, identity_bf, kT, vp, qT_tiles):
    with (
        tc.tile_pool(name="natb", bufs=3) as npool,
        tc.tile_pool(name="workb", bufs=2) as wpool,
        tc.tile_pool(name="psb", bufs=2, space="PSUM") as pspool,
        tc.tile_pool(name="ps1b", bufs=1, space="PSUM") as ps1pool,
    ):
            # table preload: 1-element exp so ACT_TABLE_LOAD overlaps the DMAs
            warm = wpool.tile([1, 1], F32, tag="warm", bufs=1, name="warm")
            nc.scalar.activation(
                warm[:], identity[0:1, 0:1], mybir.ActivationFunctionType.Exp
            )

            def cast_group(dst_bf, src_nat, g):
                nc.vector.tensor_copy(
                    dst_bf[:, 4 * g : 4 * (g + 1), :], src_nat[:, 4 * g : 4 * (g + 1), :]
                )

            def transpose_group(dst, src_bf, g, tag, name):
                # PE-transpose 4 [128,64] bf16 chunks into one PSUM staging
                # bank, then one DVE copy into [64, 512] of the bf16 dst.
                pst = ps1pool.tile([D, 4, P], BF16, tag=tag, name=name)
                for t in range(4):
                    nc.tensor.transpose(
                        pst[:, t, :], src_bf[:, 4 * g + t, :], identity_bf
                    )
                nc.vector.tensor_copy(dst[0:D, 512 * g : 512 * (g + 1)], pst[:])

            def load_q(h):
                q_nat = npool.tile([P, NJ, D], F32, tag="nat", name=f"q_nat{h}")
                src = q[h].rearrange("(p c) d -> p c d", p=P)
                for g in range(4):
                    nc.sync.dma_start(
                        q_nat[:, 4 * g : 4 * (g + 1), :], src[:, 4 * g : 4 * (g + 1), :]
                    )
                return q_nat

            def qbf_tile(h):
                return wpool.tile([P, NJ, D], BF16, tag="qbf", name=f"q_bf{h}")

            # ---- startup: K/Q0 DMA pieces interleaved (k0,q0,k1,q1,...) so
            # the q-side transpose chain starts as early as the k-side; the
            # remaining transpose groups interleave with unit 0's QK groups ----
            k_nat = npool.tile([P, NJ, D], F32, tag="nat", name="k_nat")
            q_nat_next = npool.tile([P, NJ, D], F32, tag="nat", name="q_nat0")
            ksrc = k.rearrange("(p c) d -> p c d", p=P)
            qsrc = q[0].rearrange("(p c) d -> p c d", p=P)
            # 1-row dummy DMA absorbs the DMA-queue init cost before the
            # real input pieces land on the queue
            dwarm = wpool.tile([1, D], F32, tag="dwarm", bufs=1, name="dwarm")
            nc.sync.dma_start(dwarm[:], k[0:1, :])
            for g in range(4):
                gs = slice(4 * g, 4 * (g + 1))
                nc.sync.dma_start(k_nat[:, gs, :], ksrc[:, gs, :])
                nc.sync.dma_start(q_nat_next[:, gs, :], qsrc[:, gs, :])
            v_nat = npool.tile([P, NJ, D], F32, tag="nat", name="v_nat")
            nc.sync.dma_start(v_nat[:], v.rearrange("(p c) d -> p c d", p=P))
            k_bf = wpool.tile([P, NJ, D], BF16, tag="kbf", bufs=1, name="k_bf")
            q_bf_cur = qbf_tile(0)
            q_nat0, q_bf0 = q_nat_next, q_bf_cur
            cast_group(k_bf, k_nat, 0)
            cast_group(q_bf_cur, q_nat_next, 0)
            transpose_group(kT, k_bf, 0, "tr", "pst_k0")
            transpose_group(qT_tiles[0], q_bf_cur, 0, "pv", "pst_q0_0")

            def make_epilogue(h, qb, pv, split_dma=False):
                def emit():
                    oev = wpool.tile([D + 1, QB], BF16, tag="oev", name=f"oev{h}_{qb}")
                    nc.vector.tensor_copy(oev[:], pv[:])
                    otr = ps1pool.tile([P, 4, D + 4], BF16, tag="tr", name=f"otr{h}_{qb}")
                    rcp = wpool.tile([P, 4], F32, tag="rcp", name=f"rcp{h}_{qb}")
                    oout = wpool.tile([P, 4, D], F32, tag="oout", name=f"oout{h}_{qb}")
                    for t in range(4):
                        nc.tensor.transpose(
                            otr[:, t, 0 : D + 1],
                            oev[:, P * t : P * (t + 1)],
                            identity_bf[0 : D + 1, 0 : D + 1],
                        )
                    nc.vector.reciprocal(rcp[:], otr[:, :, D : D + 1])
                    for t in range(4):
                        nc.vector.tensor_scalar(
                            oout[:, t, :],
                            otr[:, t, 0:D],
                            rcp[:, t : t + 1],
                            None,
                            mybir.AluOpType.mult,
                        )
                    odst = o[h].rearrange("(p c) d -> p c d", p=P)
                    if split_dma:
                        # last unit: store per-pair so the first piece flies
                        # while the remaining normalizes finish
                        for t in range(0, 4, 2):
                            nc.sync.dma_start(
                                odst[:, 4 * qb + t : 4 * qb + t + 2, :],
                                oout[:, t : t + 2, :],
                            )
                    else:
                        nc.sync.dma_start(odst[:, 4 * qb : 4 * (qb + 1), :], oout[:])
                return emit

            def make_pv_tail(pv, pT):
                def emit():
                    for c in range(PV_SPLIT, NJ):
                        nc.tensor.matmul(
                            pv[:],
                            lhsT=vp[:, c, :],
                            rhs=pT[:, QB * c : QB * (c + 1)],
                            start=False,
                            stop=(c == NJ - 1),
                        )
                return emit

            def pv_chunk(pv, pT, c):
                nc.tensor.matmul(
                    pv[:],
                    lhsT=vp[:, c, :],
                    rhs=pT[:, QB * c : QB * (c + 1)],
                    start=(c == 0),
                    stop=(c == NJ - 1),
                )

            pending_pv = None
            pending_epi = None
            q_bf_next = None
            for u in range(N_UNITS):
                h, qb = divmod(u, NQB)
                last = u == N_UNITS - 1
                qT = qT_tiles[h % 2]
                qs = qT[:, QB * qb : QB * (qb + 1)]
                prep = qb in (1, 2, 3) and h + 1 < HPC
                if qb == 1 and h + 1 < HPC:
                    q_nat_next = load_q(h + 1)
                    q_bf_next = qbf_tile(h + 1)

                pT = wpool.tile([P, NJ * QB], BF16, tag="pT", name=f"pT{h}_{qb}")
                pv = None
                pv_done = 0
                for gi, gsz in enumerate(GROUPS):
                    g0 = GCUM[gi] - gsz
                    sg = pspool.tile(
                        [P, gsz, QB],
                        F32,
                        tag="sg",
                        name=f"sg{h}_{qb}_{gi}",
                        padded_shape=[P, max(GROUPS), QB],
                    )
                    for i in range(gsz):
                        j = g0 + i
                        nc.tensor.matmul(
                            sg[:, i, :],
                            lhsT=kT[:, P * j : P * (j + 1)],
                            rhs=qs,
                            start=True,
                            stop=True,
                        )
                    if gi in EXP_OFF_GIS and not last:
                        nc.vector.tensor_scalar(
                            pT[:, QB * g0 : QB * GCUM[gi]].bitcast(mybir.dt.int16),
                            sg[:],
                            EXP_A,
                            EXP_B,
                            mybir.AluOpType.mult,
                            mybir.AluOpType.add,
                        )
                    else:
                        nc.scalar.activation(
                            pT[:, QB * g0 : QB * GCUM[gi]],
                            sg[:],
                            mybir.ActivationFunctionType.Exp,
                            scale=SCALE,
                        )
                    if gi == 0 and pending_pv is not None:
                        pending_pv()
                        pending_pv = None
                    if gi == (0 if last else 1) and pending_epi is not None:
                        pending_epi()
                        pending_epi = None
                    if last and gi >= 1:
                        # final unit: run PV progressively under the exps so
                        # only chunks 14-15 + epilogue trail the last exp
                        if pv is None:
                            pv = ps1pool.tile([D + 1, QB], F32, tag="pv", name=f"pv{h}_{qb}")
                        for c in range(pv_done, GCUM[gi]):
                            pv_chunk(pv, pT, c)
                        pv_done = GCUM[gi]
                    if u == 0 and gi <= 2:
                        # remaining K^T casts & transposes (all 16 kT chunks
                        # are needed by unit 0's own later QK groups); Q^T
                        # group 1 is needed by unit 1
                        cast_group(k_bf, k_nat, gi + 1)
                        transpose_group(
                            kT, k_bf, gi + 1, "tr" if gi % 2 else "pv", f"pst_k{gi + 1}"
                        )
                        if gi == 0:
                            cast_group(q_bf0, q_nat0, 1)
                            transpose_group(qT_tiles[0], q_bf0, 1, "tr", "pst_q0_1")
                    if u == 0 and gi == 3:
                        nc.vector.tensor_copy(vp[:, :, 0:D], v_nat[:])
                    if u in (1, 2) and gi == 2:
                        # head-0 Q^T groups 2/3 spread into the units that
                        # first need them, easing the startup congestion
                        cast_group(q_bf0, q_nat0, u + 1)
                        transpose_group(
                            qT_tiles[0], q_bf0, u + 1, "tr", f"pst_q0_{u + 1}"
                        )
                    if prep:
                        # next head's Q^T prep; head 0 preps in qb2/qb3 only
                        # (its qb1 is still busy with startup spreading)
                        pgs = (
                            {2: [(2, 0), (3, 1)], 3: [(2, 2), (3, 3)]}
                            if h == 0
                            else {1: [(2, 0)], 2: [(2, 1)], 3: [(2, 2), (3, 3)]}
                        ).get(qb, [])
                        for slot_gi, pg in pgs:
                            if gi == slot_gi:
                                cast_group(q_bf_next, q_nat_next, pg)
                                transpose_group(
                                    qT_tiles[(h + 1) % 2],
                                    q_bf_next,
                                    pg,
                                    "tr" if pg % 2 == 0 else "pv",
                                    f"pst_q{h + 1}_{pg}",
                                )

                if last:
                    for c in range(pv_done, NJ):
                        pv_chunk(pv, pT, c)
                    pending_epi = make_epilogue(h, qb, pv, split_dma=True)
                else:
                    pv = ps1pool.tile([D + 1, QB], F32, tag="pv", name=f"pv{h}_{qb}")
                    for c in range(PV_SPLIT):
                        pv_chunk(pv, pT, c)
                    pending_pv = make_pv_tail(pv, pT)
                    pending_epi = make_epilogue(h, qb, pv)
                if qb == 3:
                    q_bf_cur = q_bf_next
            pending_epi()
    nc.compile()
    return nc


def _get_module(reps=1, **cfg):
    key = (reps, tuple(sorted(cfg.items())))
    if key not in _CACHED:
        _CACHED[key] = _build_module(reps, **cfg)
    return _CACHED[key]


def make_in_maps(Q, K, V):
    """Shard full inputs into per-core input maps (core c -> batch c//4,
    heads 4*(c%4)..4*(c%4)+4)."""
    Q = np.asarray(Q, dtype=np.float32)
    K = np.asarray(K, dtype=np.float32)
    V = np.asarray(V, dtype=np.float32)
    in_maps = []
    for c in range(N_CORES):
        b = c // (N_CORES // B)
        h0 = HPC * (c % (N_CORES // B))
        in_maps.append(
            {
                "q": np.ascontiguousarray(Q[b, h0 : h0 + HPC]),
                "k": np.ascontiguousarray(K[b, 0]),
                "v": np.ascontiguousarray(V[b, 0]),
            }
        )
    return in_maps


def assemble_output(results):
    out = np.empty((B, H, S, D), dtype=np.float32)
    for c in range(N_CORES):
        b = c // (N_CORES // B)
        h0 = HPC * (c % (N_CORES // B))
        out[b, h0 : h0 + HPC] = results[c]["o"]
    return out


def kernel(Q, K, V):
    nc = _get_module(1, **DEFAULT_CFG)
    res = run_bass_kernel_spmd(nc, make_in_maps(Q, K, V), core_ids=list(range(N_CORES)))
    return assemble_output(res.results)
